# revision 20
# baseline (speedup 1.0000x reference)
"""Trainium2 Bass kernel for nn_MoELayer (moe_routing).

Strategy (data-parallel over batch, 8 NeuronCores):
  * Fold the multi-scale moving-average decomposition into the expert
    weights:  out_e[p,f] = sum_s x[s,f] * Weff[e,p,s] + bias[e,p] with
    Weff = Ws + sum_n A_n^T (Wl_n - Ws)  (A_n = reflect-pad moving avg).
    This shrinks the contraction 4096 -> 1024 and weight bytes 4x.
  * Exploit top-2-of-16 sparsity: compute the gate on device (fp32
    logits matmul -> max8/max_index -> w = sigmoid(dl)), indirect-DMA
    gather the two selected Weff rows per token (bf16), combine
    V = w1*G1 + w2*G2, PE-transpose to [S, tokens], then one bf16
    matmul per batch V @ x[b] accumulated in fp32 PSUM.
  * KL term computed on device from logits (sum log g = sum l - E*logZ),
    partial per core; host sums the 8 partial scalars.
"""

import math
import os
import sys

import numpy as np

for _p in ("/opt/trn_rl_repo",):
    if _p not in sys.path and os.path.isdir(_p):
        sys.path.append(_p)

import concourse.bass as bass
import concourse.mybir as mybir
from concourse import bacc
from concourse import bass_utils
from concourse.bass import IndirectOffsetOnAxis
from concourse.masks import make_identity
from concourse.tile import TileContext

F32 = mybir.dt.float32
BF16 = mybir.dt.bfloat16
I32 = mybir.dt.int32
U32 = mybir.dt.uint32
AF = mybir.ActivationFunctionType
ALU = mybir.AluOpType

# problem constants
B, S, F, E, P, nS = 64, 1024, 256, 16, 336, 3
SCALES = [3, 7, 14]
NFREQ = 4
MAX_TIME = 200.0
KL_LAMBDA = 1e-3
N_CORES = 8
NB = B // N_CORES          # batches per core
SAUG = S + 4               # weff rows padded: col 1024 = bias, 1025..1027 = 0
TOK = NB * P               # tokens per core (2688)
KCH = S // 128             # 8 contraction chunks
# token tiles per batch: p in [0,128), [128,256), [256,336)
M_TILES = [(0, 128), (128, 128), (256, 80)]
# x s-chunks holding the last P positions (s in [688, 1024)):
#   (chunk, col_offset_in_tokens, rows_used_from_chunk_top)
XT_CHUNKS = [(5, 0, 80), (6, 80, 128), (7, 208, 128)]
MULTIROW_GATHER = False
XBAR_TRANSPOSE = True


def _ma_matrix(n, w):
    """Dense [n,n] matrix of torch-style reflect-pad moving average."""
    lp = w // 2
    rp = lp - (1 if w % 2 == 0 else 0)
    A = np.zeros((n, n), dtype=np.float64)
    idx = np.zeros(n + lp + rp, dtype=np.int64)
    for j in range(n + lp + rp):
        if j < lp:
            idx[j] = lp - j
        elif j < lp + n:
            idx[j] = j - lp
        else:
            idx[j] = (n - 2) - (j - lp - n)
    inv = 1.0 / w
    for sp in range(n):
        for j in range(sp, sp + w):
            A[sp, idx[j]] += inv
    return A


def fold_weights(Wl, bl, Ws, bs):
    """Weff_aug [E*P, SAUG] bf16 (col S = bias, rest zero-pad)."""
    import ml_dtypes

    A = [_ma_matrix(S, w) for w in SCALES]
    Wsf = Ws.astype(np.float64)
    weff = np.array(Wsf)
    for n in range(nS):
        d = Wl[:, n, :, :].astype(np.float64) - Wsf
        # (A^T d)^T per row  ==  d @ A
        weff += (d.reshape(-1, S) @ A[n]).reshape(E, P, S)
    bias = (bl.sum(axis=1) + bs).astype(np.float64)
    aug = np.zeros((E * P, SAUG), dtype=np.float32)
    aug[:, :S] = weff.reshape(E * P, S).astype(np.float32)
    aug[:, S] = bias.reshape(E * P).astype(np.float32)
    return aug.astype(ml_dtypes.bfloat16)


def build_module():
    """Build the per-core Bass program (same program on all 8 cores)."""
    nc = bacc.Bacc("TRN2", target_bir_lowering=False)

    x_d = nc.dram_tensor("x_l", [NB, S, F], F32, kind="ExternalInput")
    hh_d = nc.dram_tensor("hh_l", [1, TOK], F32, kind="ExternalInput")
    weff_d = nc.dram_tensor("weff", [E * P, SAUG], BF16, kind="ExternalInput")
    gwt_d = nc.dram_tensor("gwt", [F + 2 * NFREQ + 1, E], F32, kind="ExternalInput")
    cb8_d = nc.dram_tensor("cb8", [8, 2], F32, kind="ExternalInput")
    out_d = nc.dram_tensor("out_l", [F, TOK], F32, kind="ExternalOutput")
    klp_d = nc.dram_tensor("klp", [1, 1], F32, kind="ExternalOutput")

    u = 1.0 / E
    kl_scale = -KL_LAMBDA * u / B
    kl_bias = KL_LAMBDA * u * (TOK * E) * math.log(u) / B

    from contextlib import ExitStack

    with TileContext(nc) as tc, ExitStack() as es:
        cpool = es.enter_context(tc.tile_pool(name="const", bufs=1))
        spool = es.enter_context(tc.tile_pool(name="smalls", bufs=3))
        wpool = es.enter_context(tc.tile_pool(name="wcoef", bufs=6))
        xfpool = es.enter_context(tc.tile_pool(name="xf", bufs=2))
        xbpool = es.enter_context(tc.tile_pool(name="xb", bufs=2))
        xtpool = es.enter_context(tc.tile_pool(name="xt", bufs=2))
        gpool = es.enter_context(tc.tile_pool(name="gath", bufs=3))
        vpool = es.enter_context(tc.tile_pool(name="vcomb", bufs=3))
        vtpool = es.enter_context(tc.tile_pool(name="vt", bufs=2))
        opool = es.enter_context(tc.tile_pool(name="osb", bufs=3))
        pp_tp = es.enter_context(tc.tile_pool(name="ps_tp", bufs=2, space="PSUM"))
        pp_lg = es.enter_context(tc.tile_pool(name="ps_lg", bufs=1, space="PSUM"))
        pp_o = es.enter_context(tc.tile_pool(name="ps_o", bufs=2, space="PSUM"))

        # ---- constants
        id_f = cpool.tile([128, 128], F32)
        make_identity(nc, id_f)
        id_b = cpool.tile([128, 128], BF16)
        make_identity(nc, id_b)
        gw0 = cpool.tile([128, E], F32)
        nc.sync.dma_start(gw0, gwt_d[0:128, :])
        gw1 = cpool.tile([128, E], F32)
        nc.sync.dma_start(gw1, gwt_d[128:256, :])
        gw2 = cpool.tile([9, E], F32)
        nc.sync.dma_start(gw2, gwt_d[256:265, :])
        cb8 = cpool.tile([8, 2], F32)
        nc.sync.dma_start(cb8, cb8_d[:, :])
        ones_c = cpool.tile([128, 1], F32)
        nc.vector.memset(ones_c, 1.0)
        ones_bf = cpool.tile([1, 128], BF16)
        nc.vector.memset(ones_bf, 1.0)
        # KL accumulators: Z per (b,m) column (init 1 -> ln=0), sum-of-logits
        zacc = cpool.tile([128, NB * 3], F32)
        nc.vector.memset(zacc, 1.0)
        slacc = cpool.tile([128, NB * 3], F32)
        nc.vector.memset(slacc, 0.0)
        pcol = []
        for m, (off, rn) in enumerate(M_TILES):
            pi = cpool.tile([128, 1], I32, tag=f"pci{m}")
            nc.gpsimd.iota(pi, pattern=[[0, 1]], base=off, channel_multiplier=1)
            pf = cpool.tile([128, 1], F32, tag=f"pcf{m}")
            nc.vector.tensor_copy(pf, pi)
            pcol.append(pf)

        # ---- stage encoding (range-reduced sin), fp32, all tokens at once
        # u8 = t*(f/2) (+0.25 for cos channels); v = u8 mod 1; a = v - (v>=.5)
        # enc = sin(2*pi*a)
        enc9 = cpool.tile([9, TOK], F32)
        nc.vector.memset(enc9, 1.0)
        for k in range(8):
            nc.sync.dma_start(enc9[k : k + 1, :], hh_d[:, :])
        u8 = cpool.tile([8, TOK], F32)
        nc.scalar.activation(
            u8, enc9[0:8, :], AF.Identity, scale=cb8[:, 0:1], bias=cb8[:, 1:2]
        )
        m1 = cpool.tile([8, TOK], F32, tag="enctmp")
        nc.vector.tensor_scalar(m1, u8, 1.0, scalar2=None, op0=ALU.is_ge)
        nc.vector.tensor_sub(u8, u8, m1)
        nc.vector.tensor_scalar(m1, u8, 1.0, scalar2=None, op0=ALU.is_ge)
        nc.vector.tensor_sub(u8, u8, m1)
        nc.vector.tensor_scalar(m1, u8, 0.5, scalar2=None, op0=ALU.is_ge)
        nc.vector.tensor_sub(u8, u8, m1)
        nc.scalar.activation(enc9[0:8, :], u8, AF.Sin, scale=2.0 * math.pi)

        # ---- main loop over this core's batches
        for b in range(NB):
            xf = xfpool.tile([128, KCH, F], F32)
            nc.sync.dma_start(
                xf, x_d[b : b + 1, :, :].rearrange("o (k p) f -> p (o k) f", p=128)
            )
            xb16 = xbpool.tile([128, KCH, F], BF16)
            nc.vector.tensor_copy(xb16, xf)

            # transpose the gating slice of x: xt[f, fc, tokens]
            xt = xtpool.tile([128, 2, P], F32)
            for (kch, c0, rows) in XT_CHUNKS:
                for fc in range(2):
                    pt = pp_tp.tile([128, 128], F32, tag="tp")
                    nc.tensor.transpose(
                        out=pt,
                        in_=xf[:, kch, fc * 128 : (fc + 1) * 128],
                        identity=id_f,
                    )
                    nc.vector.tensor_copy(
                        xt[:, fc, c0 : c0 + rows], pt[:, 128 - rows : 128]
                    )

            # ---- logits for the whole batch, transposed: plT[e, tok]
            plT = pp_lg.tile([E, P], F32, tag="lgT")
            nc.tensor.matmul(plT, lhsT=gw0, rhs=xt[:, 0, :], start=True, stop=False)
            nc.tensor.matmul(plT, lhsT=gw1, rhs=xt[:, 1, :], start=False, stop=False)
            nc.tensor.matmul(
                plT, lhsT=gw2, rhs=enc9[:, b * P : (b + 1) * P],
                start=False, stop=True,
            )
            LT = spool.tile([E, P], F32, tag="LT")
            nc.vector.tensor_copy(LT, plT)

            vtb = vtpool.tile([128, KCH + 1, P], BF16)
            for m, (off, rn) in enumerate(M_TILES):
                # ---- logits back to [tok, e]
                pl = pp_lg.tile([128, E], F32, tag="lg")
                nc.tensor.transpose(
                    out=pl[:rn], in_=LT[:, off : off + rn], identity=id_f[:E, :E]
                )
                L = spool.tile([128, E], F32, tag="L")
                nc.vector.tensor_copy(L[:rn], pl[:rn])

                # ---- KL pieces (Ln deferred to the tail; one table set here)
                col = b * 3 + m
                eZ = spool.tile([128, E], F32, tag="eZ")
                nc.scalar.activation(
                    eZ[:rn], L[:rn], AF.Exp, accum_out=zacc[:rn, col : col + 1]
                )
                nc.vector.tensor_reduce(
                    slacc[:rn, col : col + 1], L[:rn],
                    axis=mybir.AxisListType.X, op=ALU.add,
                )

                # ---- top-2; gate weights via exp ratio (no sigmoid table)
                M8 = spool.tile([128, 8], F32, tag="M8")
                nc.vector.max(M8[:rn], L[:rn])
                I8 = spool.tile([128, 8], U32, tag="I8")
                nc.vector.max_index(I8[:rn], M8[:rn], L[:rn])
                E2 = spool.tile([128, 2], F32, tag="E2")
                nc.scalar.activation(E2[:rn], M8[:rn, 0:2], AF.Exp)
                s12 = spool.tile([128, 1], F32, tag="s12")
                nc.vector.tensor_add(s12[:rn], E2[:rn, 0:1], E2[:rn, 1:2])
                r12 = spool.tile([128, 1], F32, tag="r12")
                nc.vector.reciprocal(r12[:rn], s12[:rn])
                w1 = wpool.tile([128, 1], F32, tag="w1")
                nc.vector.tensor_mul(w1[:rn], E2[:rn, 0:1], r12[:rn])
                w2 = wpool.tile([128, 1], F32, tag="w2")
                nc.vector.tensor_mul(w2[:rn], E2[:rn, 1:2], r12[:rn])

                # ---- weff row ids: idx = e*P + p
                If2 = spool.tile([128, 2], F32, tag="If2")
                nc.vector.tensor_copy(If2[:rn], I8[:rn, 0:2])
                idxf = spool.tile([128, 2], F32, tag="idxf")
                nc.vector.scalar_tensor_tensor(
                    idxf[:rn], If2[:rn], float(P),
                    pcol[m].to_broadcast([128, 2])[:rn], op0=ALU.mult, op1=ALU.add,
                )
                idxi = spool.tile([128, 2], I32, tag="idxi")
                nc.vector.tensor_copy(idxi[:rn], idxf[:rn])

                # ---- gather the two expert rows
                G12 = gpool.tile([128, 2, SAUG], BF16, tag="G12")
                if MULTIROW_GATHER:
                    nc.gpsimd.indirect_dma_start(
                        out=G12[:rn], out_offset=None, in_=weff_d[:, :],
                        in_offset=IndirectOffsetOnAxis(ap=idxi[:rn, 0:2], axis=0),
                    )
                else:
                    nc.gpsimd.indirect_dma_start(
                        out=G12[:rn, 0, :], out_offset=None, in_=weff_d[:, :],
                        in_offset=IndirectOffsetOnAxis(ap=idxi[:rn, 0:1], axis=0),
                    )
                    nc.gpsimd.indirect_dma_start(
                        out=G12[:rn, 1, :], out_offset=None, in_=weff_d[:, :],
                        in_offset=IndirectOffsetOnAxis(ap=idxi[:rn, 1:2], axis=0),
                    )

                # ---- V = w1*G1 + w2*G2  (bf16)
                vtmp = vpool.tile([128, SAUG], BF16, tag="vtmp")
                nc.scalar.activation(vtmp[:rn], G12[:rn, 0, :], AF.Copy, scale=w1[:rn])
                V = vpool.tile([128, SAUG], BF16, tag="V")
                nc.vector.scalar_tensor_tensor(
                    V[:rn], G12[:rn, 1, :], w2[:rn], vtmp[:rn],
                    op0=ALU.mult, op1=ALU.add,
                )

                # ---- transpose V -> vtb[:, k, off:off+rn]; chunk 8 row0 = bias
                for k in range(KCH):
                    if XBAR_TRANSPOSE:
                        nc.scalar.dma_start_transpose(
                            out=vtb[:, k, off : off + rn],
                            in_=V[:rn, k * 128 : (k + 1) * 128],
                        )
                    else:
                        ptv = pp_tp.tile([128, 128], BF16, tag="tpb")
                        nc.tensor.transpose(
                            out=ptv[:, :rn],
                            in_=V[:rn, k * 128 : (k + 1) * 128],
                            identity=id_b[:rn, :rn],
                        )
                        nc.vector.tensor_copy(
                            vtb[:, k, off : off + rn], ptv[:, :rn]
                        )
                ptb = pp_tp.tile([128, 128], BF16, tag="tpb")
                nc.tensor.transpose(
                    out=ptb[:4, :rn], in_=V[:rn, S : S + 4],
                    identity=id_b[:rn, :rn],
                )
                nc.vector.tensor_copy(vtb[0:1, KCH, off : off + rn], ptb[0:1, :rn])

            # ---- main matmul: outT[f, tok] = x[b].T-chunks @ VT (+ bias row)
            for fc in range(2):
                po = pp_o.tile([128, P], F32, tag="po")
                for k in range(KCH):
                    nc.tensor.matmul(
                        po, lhsT=xb16[:, k, fc * 128 : (fc + 1) * 128],
                        rhs=vtb[:, k, :],
                        start=(k == 0), stop=False,
                    )
                nc.tensor.matmul(
                    po, lhsT=ones_bf[0:1, 0:128], rhs=vtb[0:1, KCH, :],
                    start=False, stop=True,
                )
                osb = opool.tile([128, P], F32, tag="osb")
                nc.vector.tensor_copy(osb, po)
                nc.sync.dma_start(
                    out_d[fc * 128 : (fc + 1) * 128, b * P : (b + 1) * P], osb
                )

        # ---- KL tail: klp = kl_scale * (sum slacc - E*sum ln zacc) + kl_bias
        ln24 = cpool.tile([128, NB * 3], F32)
        nc.scalar.activation(ln24, zacc, AF.Ln)
        kacc = cpool.tile([128, NB * 3], F32)
        nc.vector.scalar_tensor_tensor(
            kacc, ln24, -float(E), slacc, op0=ALU.mult, op1=ALU.add
        )
        kc = cpool.tile([128, 1], F32)
        nc.vector.tensor_reduce(kc, kacc, axis=mybir.AxisListType.X, op=ALU.add)
        pk = pp_lg.tile([1, 1], F32, tag="lg")
        nc.tensor.matmul(pk, lhsT=ones_c, rhs=kc, start=True, stop=True)
        kb = cpool.tile([1, 1], F32)
        nc.vector.memset(kb, kl_bias)
        ks = cpool.tile([1, 1], F32)
        nc.scalar.activation(ks, pk, AF.Identity, scale=kl_scale, bias=kb[:, :])
        nc.sync.dma_start(klp_d[:, :], ks)

    nc.compile()
    return nc


_CACHE = {}


def _prep_inputs(x, x_mark_enc, gate_w, gate_b, Wl, bl, Ws, bs):
    weff = fold_weights(Wl, bl, Ws, bs)
    gwt = np.zeros((F + 2 * NFREQ + 1, E), dtype=np.float32)
    gwt[: F + 2 * NFREQ, :] = gate_w.T
    gwt[F + 2 * NFREQ, :] = gate_b
    freqs = np.arange(1, NFREQ + 1, dtype=np.float32)
    cb8 = np.zeros((8, 2), dtype=np.float32)
    cb8[:4, 0] = freqs / (2.0 * MAX_TIME)
    cb8[4:, 0] = freqs / (2.0 * MAX_TIME)
    cb8[4:, 1] = 0.25
    hh = np.ascontiguousarray(x_mark_enc[:, S - P :, -1], dtype=np.float32)  # [B,P]
    in_maps = []
    for c in range(N_CORES):
        in_maps.append(
            {
                "x_l": np.ascontiguousarray(x[c * NB : (c + 1) * NB]),
                "hh_l": hh[c * NB : (c + 1) * NB].reshape(1, TOK),
                "weff": weff,
                "gwt": gwt,
                "cb8": cb8,
            }
        )
    return in_maps


def kernel(x, x_mark_enc, gate_w, gate_b, Wl, bl, Ws, bs, trace=False):
    if "nc" not in _CACHE:
        _CACHE["nc"] = build_module()
    nc = _CACHE["nc"]
    in_maps = _prep_inputs(x, x_mark_enc, gate_w, gate_b, Wl, bl, Ws, bs)
    res = bass_utils.run_bass_kernel_spmd(
        nc, in_maps, core_ids=list(range(N_CORES)), trace=trace
    )
    _CACHE["last_result"] = res
    out = np.concatenate(
        [np.ascontiguousarray(r["out_l"].T).reshape(NB, P, F) for r in res.results],
        axis=0,
    ).astype(np.float32)
    kl = np.float32(sum(float(r["klp"][0, 0]) for r in res.results))
    return out, kl


# revision 21
# speedup vs baseline: 2.2399x; 2.2399x over previous
"""Trainium2 Bass kernel for nn_MoELayer (moe_routing).

Strategy (data-parallel over batch, 8 NeuronCores):
  * Fold the multi-scale moving-average decomposition into the expert
    weights:  out_e[p,f] = sum_s x[s,f] * Weff[e,p,s] + bias[e,p] with
    Weff = Ws + sum_n A_n^T (Wl_n - Ws)  (A_n = reflect-pad moving avg).
    This shrinks the contraction 4096 -> 1024 and weight bytes 4x.
  * Exploit top-2-of-16 sparsity: compute the gate on device (fp32
    logits matmul -> max8/max_index -> w = sigmoid(dl)), indirect-DMA
    gather the two selected Weff rows per token (bf16), combine
    V = w1*G1 + w2*G2, PE-transpose to [S, tokens], then one bf16
    matmul per batch V @ x[b] accumulated in fp32 PSUM.
  * KL term computed on device from logits (sum log g = sum l - E*logZ),
    partial per core; host sums the 8 partial scalars.
"""

import math
import os
import sys

import numpy as np

for _p in ("/opt/trn_rl_repo",):
    if _p not in sys.path and os.path.isdir(_p):
        sys.path.append(_p)

import concourse.bass as bass
import concourse.mybir as mybir
from concourse import bacc
from concourse import bass_utils
from concourse.bass import IndirectOffsetOnAxis
from concourse.masks import make_identity
from concourse.tile import TileContext

F32 = mybir.dt.float32
BF16 = mybir.dt.bfloat16
I32 = mybir.dt.int32
U32 = mybir.dt.uint32
AF = mybir.ActivationFunctionType
ALU = mybir.AluOpType

# problem constants
B, S, F, E, P, nS = 64, 1024, 256, 16, 336, 3
SCALES = [3, 7, 14]
NFREQ = 4
MAX_TIME = 200.0
KL_LAMBDA = 1e-3
N_CORES = 8
NB = B // N_CORES          # batches per core
SAUG = S + 4               # weff rows padded: col 1024 = bias, 1025..1027 = 0
TOK = NB * P               # tokens per core (2688)
KCH = S // 128             # 8 contraction chunks
# token tiles per batch: p in [0,128), [128,256), [256,336)
M_TILES = [(0, 128), (128, 128), (256, 80)]
# x s-chunks holding the last P positions (s in [688, 1024)):
#   (chunk, col_offset_in_tokens, rows_used_from_chunk_top)
XT_CHUNKS = [(5, 0, 80), (6, 80, 128), (7, 208, 128)]
MULTIROW_GATHER = False
XBAR_TRANSPOSE = False


def _ma_matrix(n, w):
    """Dense [n,n] matrix of torch-style reflect-pad moving average."""
    lp = w // 2
    rp = lp - (1 if w % 2 == 0 else 0)
    A = np.zeros((n, n), dtype=np.float64)
    idx = np.zeros(n + lp + rp, dtype=np.int64)
    for j in range(n + lp + rp):
        if j < lp:
            idx[j] = lp - j
        elif j < lp + n:
            idx[j] = j - lp
        else:
            idx[j] = (n - 2) - (j - lp - n)
    inv = 1.0 / w
    for sp in range(n):
        for j in range(sp, sp + w):
            A[sp, idx[j]] += inv
    return A


def fold_weights(Wl, bl, Ws, bs):
    """Weff_aug [E*P, SAUG] bf16 (col S = bias, rest zero-pad)."""
    import ml_dtypes

    A = [_ma_matrix(S, w) for w in SCALES]
    Wsf = Ws.astype(np.float64)
    weff = np.array(Wsf)
    for n in range(nS):
        d = Wl[:, n, :, :].astype(np.float64) - Wsf
        # (A^T d)^T per row  ==  d @ A
        weff += (d.reshape(-1, S) @ A[n]).reshape(E, P, S)
    bias = (bl.sum(axis=1) + bs).astype(np.float64)
    aug = np.zeros((E * P, SAUG), dtype=np.float32)
    aug[:, :S] = weff.reshape(E * P, S).astype(np.float32)
    aug[:, S] = bias.reshape(E * P).astype(np.float32)
    return aug.astype(ml_dtypes.bfloat16)


def build_module():
    """Build the per-core Bass program (same program on all 8 cores)."""
    nc = bacc.Bacc("TRN2", target_bir_lowering=False)

    x_d = nc.dram_tensor("x_l", [NB, S, F], F32, kind="ExternalInput")
    hh_d = nc.dram_tensor("hh_l", [1, TOK], F32, kind="ExternalInput")
    weff_d = nc.dram_tensor("weff", [E * P, SAUG], BF16, kind="ExternalInput")
    gwt_d = nc.dram_tensor("gwt", [F + 2 * NFREQ + 1, E], F32, kind="ExternalInput")
    cb8_d = nc.dram_tensor("cb8", [8, 2], F32, kind="ExternalInput")
    out_d = nc.dram_tensor("out_l", [F, TOK], F32, kind="ExternalOutput")
    klp_d = nc.dram_tensor("klp", [1, 1], F32, kind="ExternalOutput")

    u = 1.0 / E
    kl_scale = -KL_LAMBDA * u / B
    kl_bias = KL_LAMBDA * u * (TOK * E) * math.log(u) / B

    from contextlib import ExitStack

    with TileContext(nc) as tc, ExitStack() as es:
        cpool = es.enter_context(tc.tile_pool(name="const", bufs=1))
        spool = es.enter_context(tc.tile_pool(name="smalls", bufs=3))
        wpool = es.enter_context(tc.tile_pool(name="wcoef", bufs=6))
        xfpool = es.enter_context(tc.tile_pool(name="xf", bufs=2))
        xbpool = es.enter_context(tc.tile_pool(name="xb", bufs=2))
        xtpool = es.enter_context(tc.tile_pool(name="xt", bufs=2))
        gpool = es.enter_context(tc.tile_pool(name="gath", bufs=3))
        vpool = es.enter_context(tc.tile_pool(name="vcomb", bufs=3))
        vtpool = es.enter_context(tc.tile_pool(name="vt", bufs=2))
        opool = es.enter_context(tc.tile_pool(name="osb", bufs=3))
        pp_tp = es.enter_context(tc.tile_pool(name="ps_tp", bufs=2, space="PSUM"))
        pp_lg = es.enter_context(tc.tile_pool(name="ps_lg", bufs=1, space="PSUM"))
        pp_o = es.enter_context(tc.tile_pool(name="ps_o", bufs=2, space="PSUM"))

        # ---- constants
        id_f = cpool.tile([128, 128], F32)
        make_identity(nc, id_f)
        id_b = cpool.tile([128, 128], BF16)
        make_identity(nc, id_b)
        gw0 = cpool.tile([128, E], F32)
        nc.sync.dma_start(gw0, gwt_d[0:128, :])
        gw1 = cpool.tile([128, E], F32)
        nc.sync.dma_start(gw1, gwt_d[128:256, :])
        gw2 = cpool.tile([9, E], F32)
        nc.sync.dma_start(gw2, gwt_d[256:265, :])
        cb8 = cpool.tile([8, 2], F32)
        nc.sync.dma_start(cb8, cb8_d[:, :])
        ones_c = cpool.tile([128, 1], F32)
        nc.vector.memset(ones_c, 1.0)
        ones_bf = cpool.tile([1, 128], BF16)
        nc.vector.memset(ones_bf, 1.0)
        # KL accumulators: Z per (b,m) column (init 1 -> ln=0), sum-of-logits
        zacc = cpool.tile([128, NB * 3], F32)
        nc.vector.memset(zacc, 1.0)
        slacc = cpool.tile([128, NB * 3], F32)
        nc.vector.memset(slacc, 0.0)
        pcol = []
        for m, (off, rn) in enumerate(M_TILES):
            pi = cpool.tile([128, 1], I32, tag=f"pci{m}")
            nc.gpsimd.iota(pi, pattern=[[0, 1]], base=off, channel_multiplier=1)
            pf = cpool.tile([128, 1], F32, tag=f"pcf{m}")
            nc.vector.tensor_copy(pf, pi)
            pcol.append(pf)

        # ---- stage encoding (range-reduced sin), fp32, all tokens at once
        # u8 = t*(f/2) (+0.25 for cos channels); v = u8 mod 1; a = v - (v>=.5)
        # enc = sin(2*pi*a)
        enc9 = cpool.tile([9, TOK], F32)
        nc.vector.memset(enc9, 1.0)
        for k in range(8):
            nc.sync.dma_start(enc9[k : k + 1, :], hh_d[:, :])
        u8 = cpool.tile([8, TOK], F32)
        nc.scalar.activation(
            u8, enc9[0:8, :], AF.Identity, scale=cb8[:, 0:1], bias=cb8[:, 1:2]
        )
        m1 = cpool.tile([8, TOK], F32, tag="enctmp")
        nc.vector.tensor_scalar(m1, u8, 1.0, scalar2=None, op0=ALU.is_ge)
        nc.vector.tensor_sub(u8, u8, m1)
        nc.vector.tensor_scalar(m1, u8, 1.0, scalar2=None, op0=ALU.is_ge)
        nc.vector.tensor_sub(u8, u8, m1)
        nc.vector.tensor_scalar(m1, u8, 0.5, scalar2=None, op0=ALU.is_ge)
        nc.vector.tensor_sub(u8, u8, m1)
        nc.scalar.activation(enc9[0:8, :], u8, AF.Sin, scale=2.0 * math.pi)

        # ---- main loop over this core's batches
        for b in range(NB):
            xf = xfpool.tile([128, KCH, F], F32)
            nc.sync.dma_start(
                xf, x_d[b : b + 1, :, :].rearrange("o (k p) f -> p (o k) f", p=128)
            )
            xb16 = xbpool.tile([128, KCH, F], BF16)
            nc.vector.tensor_copy(xb16, xf)

            # transpose the gating slice of x: xt[f, fc, tokens]
            xt = xtpool.tile([128, 2, P], F32)
            for (kch, c0, rows) in XT_CHUNKS:
                for fc in range(2):
                    pt = pp_tp.tile([128, 128], F32, tag="tp")
                    nc.tensor.transpose(
                        out=pt,
                        in_=xf[:, kch, fc * 128 : (fc + 1) * 128],
                        identity=id_f,
                    )
                    nc.vector.tensor_copy(
                        xt[:, fc, c0 : c0 + rows], pt[:, 128 - rows : 128]
                    )

            # ---- logits for the whole batch, transposed: plT[e, tok]
            plT = pp_lg.tile([E, P], F32, tag="lgT")
            nc.tensor.matmul(plT, lhsT=gw0, rhs=xt[:, 0, :], start=True, stop=False)
            nc.tensor.matmul(plT, lhsT=gw1, rhs=xt[:, 1, :], start=False, stop=False)
            nc.tensor.matmul(
                plT, lhsT=gw2, rhs=enc9[:, b * P : (b + 1) * P],
                start=False, stop=True,
            )
            LT = spool.tile([E, P], F32, tag="LT")
            nc.vector.tensor_copy(LT, plT)

            vtb = vtpool.tile([128, KCH + 1, P], BF16)
            for m, (off, rn) in enumerate(M_TILES):
                # ---- logits back to [tok, e]
                pl = pp_lg.tile([128, E], F32, tag="lg")
                nc.tensor.transpose(
                    out=pl[:rn], in_=LT[:, off : off + rn], identity=id_f[:E, :E]
                )
                L = spool.tile([128, E], F32, tag="L")
                nc.vector.tensor_copy(L[:rn], pl[:rn])

                # ---- KL pieces (Ln deferred to the tail; one table set here)
                col = b * 3 + m
                eZ = spool.tile([128, E], F32, tag="eZ")
                nc.scalar.activation(
                    eZ[:rn], L[:rn], AF.Exp, accum_out=zacc[:rn, col : col + 1]
                )
                nc.vector.tensor_reduce(
                    slacc[:rn, col : col + 1], L[:rn],
                    axis=mybir.AxisListType.X, op=ALU.add,
                )

                # ---- top-2; gate weights via exp ratio (no sigmoid table)
                M8 = spool.tile([128, 8], F32, tag="M8")
                nc.vector.max(M8[:rn], L[:rn])
                I8 = spool.tile([128, 8], U32, tag="I8")
                nc.vector.max_index(I8[:rn], M8[:rn], L[:rn])
                E2 = spool.tile([128, 2], F32, tag="E2")
                nc.scalar.activation(E2[:rn], M8[:rn, 0:2], AF.Exp)
                s12 = spool.tile([128, 1], F32, tag="s12")
                nc.vector.tensor_add(s12[:rn], E2[:rn, 0:1], E2[:rn, 1:2])
                r12 = spool.tile([128, 1], F32, tag="r12")
                nc.vector.reciprocal(r12[:rn], s12[:rn])
                w1 = wpool.tile([128, 1], F32, tag="w1")
                nc.vector.tensor_mul(w1[:rn], E2[:rn, 0:1], r12[:rn])
                w2 = wpool.tile([128, 1], F32, tag="w2")
                nc.vector.tensor_mul(w2[:rn], E2[:rn, 1:2], r12[:rn])

                # ---- weff row ids: idx = e*P + p
                If2 = spool.tile([128, 2], F32, tag="If2")
                nc.vector.tensor_copy(If2[:rn], I8[:rn, 0:2])
                idxf = spool.tile([128, 2], F32, tag="idxf")
                nc.vector.scalar_tensor_tensor(
                    idxf[:rn], If2[:rn], float(P),
                    pcol[m].to_broadcast([128, 2])[:rn], op0=ALU.mult, op1=ALU.add,
                )
                idxi = spool.tile([128, 2], I32, tag="idxi")
                nc.vector.tensor_copy(idxi[:rn], idxf[:rn])

                # ---- gather the two expert rows
                G12 = gpool.tile([128, 2, SAUG], BF16, tag="G12")
                if MULTIROW_GATHER:
                    nc.gpsimd.indirect_dma_start(
                        out=G12[:rn], out_offset=None, in_=weff_d[:, :],
                        in_offset=IndirectOffsetOnAxis(ap=idxi[:rn, 0:2], axis=0),
                    )
                else:
                    nc.gpsimd.indirect_dma_start(
                        out=G12[:rn, 0, :], out_offset=None, in_=weff_d[:, :],
                        in_offset=IndirectOffsetOnAxis(ap=idxi[:rn, 0:1], axis=0),
                    )
                    nc.gpsimd.indirect_dma_start(
                        out=G12[:rn, 1, :], out_offset=None, in_=weff_d[:, :],
                        in_offset=IndirectOffsetOnAxis(ap=idxi[:rn, 1:2], axis=0),
                    )

                # ---- V = w1*G1 + w2*G2  (bf16)
                vtmp = vpool.tile([128, SAUG], BF16, tag="vtmp")
                nc.scalar.activation(vtmp[:rn], G12[:rn, 0, :], AF.Copy, scale=w1[:rn])
                V = vpool.tile([128, SAUG], BF16, tag="V")
                nc.vector.scalar_tensor_tensor(
                    V[:rn], G12[:rn, 1, :], w2[:rn], vtmp[:rn],
                    op0=ALU.mult, op1=ALU.add,
                )

                # ---- transpose V -> vtb[:, k, off:off+rn]; chunk 8 row0 = bias
                for k in range(KCH):
                    if XBAR_TRANSPOSE:
                        nc.scalar.dma_start_transpose(
                            out=vtb[:, k, off : off + rn],
                            in_=V[:rn, k * 128 : (k + 1) * 128],
                        )
                    else:
                        ptv = pp_tp.tile([128, 128], BF16, tag="tpb")
                        nc.tensor.transpose(
                            out=ptv[:, :rn],
                            in_=V[:rn, k * 128 : (k + 1) * 128],
                            identity=id_b[:rn, :rn],
                        )
                        nc.vector.tensor_copy(
                            vtb[:, k, off : off + rn], ptv[:, :rn]
                        )
                ptb = pp_tp.tile([128, 128], BF16, tag="tpb")
                nc.tensor.transpose(
                    out=ptb[:4, :rn], in_=V[:rn, S : S + 4],
                    identity=id_b[:rn, :rn],
                )
                nc.vector.tensor_copy(vtb[0:1, KCH, off : off + rn], ptb[0:1, :rn])

            # ---- main matmul: outT[f, tok] = x[b].T-chunks @ VT (+ bias row)
            for fc in range(2):
                po = pp_o.tile([128, P], F32, tag="po")
                for k in range(KCH):
                    nc.tensor.matmul(
                        po, lhsT=xb16[:, k, fc * 128 : (fc + 1) * 128],
                        rhs=vtb[:, k, :],
                        start=(k == 0), stop=False,
                    )
                nc.tensor.matmul(
                    po, lhsT=ones_bf[0:1, 0:128], rhs=vtb[0:1, KCH, :],
                    start=False, stop=True,
                )
                osb = opool.tile([128, P], F32, tag="osb")
                nc.vector.tensor_copy(osb, po)
                nc.sync.dma_start(
                    out_d[fc * 128 : (fc + 1) * 128, b * P : (b + 1) * P], osb
                )

        # ---- KL tail: klp = kl_scale * (sum slacc - E*sum ln zacc) + kl_bias
        ln24 = cpool.tile([128, NB * 3], F32)
        nc.scalar.activation(ln24, zacc, AF.Ln)
        kacc = cpool.tile([128, NB * 3], F32)
        nc.vector.scalar_tensor_tensor(
            kacc, ln24, -float(E), slacc, op0=ALU.mult, op1=ALU.add
        )
        kc = cpool.tile([128, 1], F32)
        nc.vector.tensor_reduce(kc, kacc, axis=mybir.AxisListType.X, op=ALU.add)
        pk = pp_lg.tile([1, 1], F32, tag="lg")
        nc.tensor.matmul(pk, lhsT=ones_c, rhs=kc, start=True, stop=True)
        kb = cpool.tile([1, 1], F32)
        nc.vector.memset(kb, kl_bias)
        ks = cpool.tile([1, 1], F32)
        nc.scalar.activation(ks, pk, AF.Identity, scale=kl_scale, bias=kb[:, :])
        nc.sync.dma_start(klp_d[:, :], ks)

    nc.compile()
    return nc


_CACHE = {}


def _prep_inputs(x, x_mark_enc, gate_w, gate_b, Wl, bl, Ws, bs):
    weff = fold_weights(Wl, bl, Ws, bs)
    gwt = np.zeros((F + 2 * NFREQ + 1, E), dtype=np.float32)
    gwt[: F + 2 * NFREQ, :] = gate_w.T
    gwt[F + 2 * NFREQ, :] = gate_b
    freqs = np.arange(1, NFREQ + 1, dtype=np.float32)
    cb8 = np.zeros((8, 2), dtype=np.float32)
    cb8[:4, 0] = freqs / (2.0 * MAX_TIME)
    cb8[4:, 0] = freqs / (2.0 * MAX_TIME)
    cb8[4:, 1] = 0.25
    hh = np.ascontiguousarray(x_mark_enc[:, S - P :, -1], dtype=np.float32)  # [B,P]
    in_maps = []
    for c in range(N_CORES):
        in_maps.append(
            {
                "x_l": np.ascontiguousarray(x[c * NB : (c + 1) * NB]),
                "hh_l": hh[c * NB : (c + 1) * NB].reshape(1, TOK),
                "weff": weff,
                "gwt": gwt,
                "cb8": cb8,
            }
        )
    return in_maps


def kernel(x, x_mark_enc, gate_w, gate_b, Wl, bl, Ws, bs, trace=False):
    if "nc" not in _CACHE:
        _CACHE["nc"] = build_module()
    nc = _CACHE["nc"]
    in_maps = _prep_inputs(x, x_mark_enc, gate_w, gate_b, Wl, bl, Ws, bs)
    res = bass_utils.run_bass_kernel_spmd(
        nc, in_maps, core_ids=list(range(N_CORES)), trace=trace
    )
    _CACHE["last_result"] = res
    out = np.concatenate(
        [np.ascontiguousarray(r["out_l"].T).reshape(NB, P, F) for r in res.results],
        axis=0,
    ).astype(np.float32)
    kl = np.float32(sum(float(r["klp"][0, 0]) for r in res.results))
    return out, kl


# revision 25
# speedup vs baseline: 2.4510x; 1.0942x over previous
"""Trainium2 Bass kernel for nn_MoELayer (moe_routing).

Strategy (data-parallel over batch, 8 NeuronCores):
  * Fold the multi-scale moving-average decomposition into the expert
    weights:  out_e[p,f] = sum_s x[s,f] * Weff[e,p,s] + bias[e,p] with
    Weff = Ws + sum_n A_n^T (Wl_n - Ws)  (A_n = reflect-pad moving avg).
    This shrinks the contraction 4096 -> 1024 and weight bytes 4x.
  * Exploit top-2-of-16 sparsity: compute the gate on device (fp32
    logits matmul -> max8/max_index -> w = sigmoid(dl)), indirect-DMA
    gather the two selected Weff rows per token (bf16), combine
    V = w1*G1 + w2*G2, PE-transpose to [S, tokens], then one bf16
    matmul per batch V @ x[b] accumulated in fp32 PSUM.
  * KL term computed on device from logits (sum log g = sum l - E*logZ),
    partial per core; host sums the 8 partial scalars.
"""

import math
import os
import sys

import numpy as np

for _p in ("/opt/trn_rl_repo",):
    if _p not in sys.path and os.path.isdir(_p):
        sys.path.append(_p)

import concourse.bass as bass
import concourse.mybir as mybir
from concourse import bacc
from concourse import bass_utils
from concourse.bass import IndirectOffsetOnAxis
from concourse.masks import make_identity
from concourse.tile import TileContext

F32 = mybir.dt.float32
BF16 = mybir.dt.bfloat16
I32 = mybir.dt.int32
U32 = mybir.dt.uint32
AF = mybir.ActivationFunctionType
ALU = mybir.AluOpType

# problem constants
B, S, F, E, P, nS = 64, 1024, 256, 16, 336, 3
SCALES = [3, 7, 14]
NFREQ = 4
MAX_TIME = 200.0
KL_LAMBDA = 1e-3
N_CORES = 8
NB = B // N_CORES          # batches per core
SAUG = S + 4               # weff rows padded: col 1024 = bias, 1025..1027 = 0
TOK = NB * P               # tokens per core (2688)
KCH = S // 128             # 8 contraction chunks
# token tiles per batch: p in [0,128), [128,256), [256,336)
M_TILES = [(0, 128), (128, 128), (256, 80)]
# x s-chunks holding the last P positions (s in [688, 1024)):
#   (chunk, col_offset_in_tokens, rows_used_from_chunk_top)
XT_CHUNKS = [(5, 0, 80), (6, 80, 128), (7, 208, 128)]
MULTIROW_GATHER = False
XBAR_TRANSPOSE = False


def _ma_matrix(n, w):
    """Dense [n,n] matrix of torch-style reflect-pad moving average."""
    lp = w // 2
    rp = lp - (1 if w % 2 == 0 else 0)
    A = np.zeros((n, n), dtype=np.float64)
    idx = np.zeros(n + lp + rp, dtype=np.int64)
    for j in range(n + lp + rp):
        if j < lp:
            idx[j] = lp - j
        elif j < lp + n:
            idx[j] = j - lp
        else:
            idx[j] = (n - 2) - (j - lp - n)
    inv = 1.0 / w
    for sp in range(n):
        for j in range(sp, sp + w):
            A[sp, idx[j]] += inv
    return A


def fold_weights(Wl, bl, Ws, bs):
    """Weff_aug [E*P, SAUG] bf16 (col S = bias, rest zero-pad)."""
    import ml_dtypes

    A = [_ma_matrix(S, w) for w in SCALES]
    Wsf = Ws.astype(np.float64)
    weff = np.array(Wsf)
    for n in range(nS):
        d = Wl[:, n, :, :].astype(np.float64) - Wsf
        # (A^T d)^T per row  ==  d @ A
        weff += (d.reshape(-1, S) @ A[n]).reshape(E, P, S)
    bias = (bl.sum(axis=1) + bs).astype(np.float64)
    aug = np.zeros((E * P, SAUG), dtype=np.float32)
    aug[:, :S] = weff.reshape(E * P, S).astype(np.float32)
    aug[:, S] = bias.reshape(E * P).astype(np.float32)
    return aug.astype(ml_dtypes.bfloat16)


def build_module():
    """Build the per-core Bass program (same program on all 8 cores)."""
    nc = bacc.Bacc("TRN2", target_bir_lowering=False)

    x_d = nc.dram_tensor("x_l", [NB, S, F], F32, kind="ExternalInput")
    hh_d = nc.dram_tensor("hh_l", [1, TOK], F32, kind="ExternalInput")
    weff_d = nc.dram_tensor("weff", [E * P, SAUG], BF16, kind="ExternalInput")
    gwt_d = nc.dram_tensor("gwt", [F + 2 * NFREQ + 1, E], F32, kind="ExternalInput")
    cb8_d = nc.dram_tensor("cb8", [8, 2], F32, kind="ExternalInput")
    out_d = nc.dram_tensor("out_l", [F, TOK], F32, kind="ExternalOutput")
    klp_d = nc.dram_tensor("klp", [1, 1], F32, kind="ExternalOutput")

    u = 1.0 / E
    kl_scale = -KL_LAMBDA * u / B
    kl_bias = KL_LAMBDA * u * (TOK * E) * math.log(u) / B

    from contextlib import ExitStack

    with TileContext(nc) as tc, ExitStack() as es:
        cpool = es.enter_context(tc.tile_pool(name="const", bufs=1))
        spool = es.enter_context(tc.tile_pool(name="smalls", bufs=3))
        wpool = es.enter_context(tc.tile_pool(name="wcoef", bufs=26))
        xfpool = es.enter_context(tc.tile_pool(name="xf", bufs=2))
        xbpool = es.enter_context(tc.tile_pool(name="xb", bufs=1))
        xtpool = es.enter_context(tc.tile_pool(name="xt", bufs=2))
        gpool = es.enter_context(tc.tile_pool(name="gath", bufs=3))
        vpool = es.enter_context(tc.tile_pool(name="vcomb", bufs=3))
        vtpool = es.enter_context(tc.tile_pool(name="vt", bufs=2))
        opool = es.enter_context(tc.tile_pool(name="osb", bufs=3))
        pp_tp = es.enter_context(tc.tile_pool(name="ps_tp", bufs=2, space="PSUM"))
        pp_lg = es.enter_context(tc.tile_pool(name="ps_lg", bufs=1, space="PSUM"))
        pp_o = es.enter_context(tc.tile_pool(name="ps_o", bufs=2, space="PSUM"))

        # ---- constants
        id_f = cpool.tile([128, 128], F32)
        make_identity(nc, id_f)
        id_b = cpool.tile([128, 128], BF16)
        make_identity(nc, id_b)
        gw0 = cpool.tile([128, E], F32)
        nc.sync.dma_start(gw0, gwt_d[0:128, :])
        gw1 = cpool.tile([128, E], F32)
        nc.sync.dma_start(gw1, gwt_d[128:256, :])
        gw2 = cpool.tile([9, E], F32)
        nc.sync.dma_start(gw2, gwt_d[256:265, :])
        cb8 = cpool.tile([8, 2], F32)
        nc.sync.dma_start(cb8, cb8_d[:, :])
        ones_c = cpool.tile([128, 1], F32)
        nc.vector.memset(ones_c, 1.0)
        ones_bf = cpool.tile([1, 128], BF16)
        nc.vector.memset(ones_bf, 1.0)
        # KL accumulators: Z per (b,m) column (init 1 -> ln=0), sum-of-logits
        zacc = cpool.tile([128, NB * 3], F32)
        nc.vector.memset(zacc, 1.0)
        slacc = cpool.tile([128, NB * 3], F32)
        nc.vector.memset(slacc, 0.0)
        pcol = []
        for m, (off, rn) in enumerate(M_TILES):
            pi = cpool.tile([128, 1], I32, tag=f"pci{m}")
            nc.gpsimd.iota(pi, pattern=[[0, 1]], base=off, channel_multiplier=1)
            pf = cpool.tile([128, 1], F32, tag=f"pcf{m}")
            nc.vector.tensor_copy(pf, pi)
            pcol.append(pf)

        # ---- stage encoding (range-reduced sin), fp32, all tokens at once
        # u8 = t*(f/2) (+0.25 for cos channels); v = u8 mod 1; a = v - (v>=.5)
        # enc = sin(2*pi*a)
        enc9 = cpool.tile([9, TOK], F32)
        nc.vector.memset(enc9, 1.0)
        for k in range(8):
            nc.sync.dma_start(enc9[k : k + 1, :], hh_d[:, :])
        u8 = cpool.tile([8, TOK], F32)
        nc.scalar.activation(
            u8, enc9[0:8, :], AF.Identity, scale=cb8[:, 0:1], bias=cb8[:, 1:2]
        )
        m1 = cpool.tile([8, TOK], F32, tag="enctmp")
        nc.vector.tensor_scalar(m1, u8, 1.0, scalar2=None, op0=ALU.is_ge)
        nc.vector.tensor_sub(u8, u8, m1)
        nc.vector.tensor_scalar(m1, u8, 1.0, scalar2=None, op0=ALU.is_ge)
        nc.vector.tensor_sub(u8, u8, m1)
        nc.vector.tensor_scalar(m1, u8, 0.5, scalar2=None, op0=ALU.is_ge)
        nc.vector.tensor_sub(u8, u8, m1)
        nc.scalar.activation(enc9[0:8, :], u8, AF.Sin, scale=2.0 * math.pi)

        # ---- pass 1: x load/cast + gating for all batches
        xb16s = []
        gate_info = []  # (b, m, off, rn, idxi, w1, w2)
        for b in range(NB):
            xf = xfpool.tile([128, KCH, F], F32)
            nc.sync.dma_start(
                xf, x_d[b : b + 1, :, :].rearrange("o (k p) f -> p (o k) f", p=128)
            )
            xb16 = xbpool.tile([128, KCH, F], BF16, tag=f"xb{b}")
            nc.vector.tensor_copy(xb16, xf)
            xb16s.append(xb16)

            # transpose the gating slice of x: xt[f, fc, tokens]
            xt = xtpool.tile([128, 2, P], F32)
            for (kch, c0, rows) in XT_CHUNKS:
                for fc in range(2):
                    pt = pp_tp.tile([128, 128], F32, tag="tp")
                    nc.tensor.transpose(
                        out=pt,
                        in_=xf[:, kch, fc * 128 : (fc + 1) * 128],
                        identity=id_f,
                    )
                    nc.vector.tensor_copy(
                        xt[:, fc, c0 : c0 + rows], pt[:, 128 - rows : 128]
                    )

            for m, (off, rn) in enumerate(M_TILES):
                # ---- logits (fp32, exact)
                pl = pp_lg.tile([128, E], F32, tag="lg")
                nc.tensor.matmul(
                    pl[:rn], lhsT=xt[:, 0, off : off + rn], rhs=gw0,
                    start=True, stop=False,
                )
                nc.tensor.matmul(
                    pl[:rn], lhsT=xt[:, 1, off : off + rn], rhs=gw1,
                    start=False, stop=False,
                )
                nc.tensor.matmul(
                    pl[:rn], lhsT=enc9[:, b * P + off : b * P + off + rn], rhs=gw2,
                    start=False, stop=True,
                )
                L = spool.tile([128, E], F32, tag="L")
                nc.vector.tensor_copy(L[:rn], pl[:rn])

                # ---- KL pieces (Ln deferred to the tail; one table set here)
                col = b * 3 + m
                eZ = spool.tile([128, E], F32, tag="eZ")
                nc.scalar.activation(
                    eZ[:rn], L[:rn], AF.Exp, accum_out=zacc[:rn, col : col + 1]
                )
                nc.vector.tensor_reduce(
                    slacc[:rn, col : col + 1], L[:rn],
                    axis=mybir.AxisListType.X, op=ALU.add,
                )

                # ---- top-2; gate weights via exp ratio (no sigmoid table)
                M8 = spool.tile([128, 8], F32, tag="M8")
                nc.vector.max(M8[:rn], L[:rn])
                I8 = spool.tile([128, 8], U32, tag="I8")
                nc.vector.max_index(I8[:rn], M8[:rn], L[:rn])
                E2 = spool.tile([128, 2], F32, tag="E2")
                nc.scalar.activation(E2[:rn], M8[:rn, 0:2], AF.Exp)
                s12 = spool.tile([128, 1], F32, tag="s12")
                nc.vector.tensor_add(s12[:rn], E2[:rn, 0:1], E2[:rn, 1:2])
                r12 = spool.tile([128, 1], F32, tag="r12")
                nc.vector.reciprocal(r12[:rn], s12[:rn])
                w1 = wpool.tile([128, 1], F32, tag="w1")
                nc.vector.tensor_mul(w1[:rn], E2[:rn, 0:1], r12[:rn])
                w2 = wpool.tile([128, 1], F32, tag="w2")
                nc.vector.tensor_mul(w2[:rn], E2[:rn, 1:2], r12[:rn])

                # ---- weff row ids: idx = e*P + p
                If2 = spool.tile([128, 2], F32, tag="If2")
                nc.vector.tensor_copy(If2[:rn], I8[:rn, 0:2])
                idxf = spool.tile([128, 2], F32, tag="idxf")
                nc.vector.scalar_tensor_tensor(
                    idxf[:rn], If2[:rn], float(P),
                    pcol[m].to_broadcast([128, 2])[:rn], op0=ALU.mult, op1=ALU.add,
                )
                idxi = wpool.tile([128, 2], I32, tag="idxi")
                nc.vector.tensor_copy(idxi[:rn], idxf[:rn])
                gate_info.append((b, m, off, rn, idxi, w1, w2))

        # ---- pass 2: gather -> combine -> transpose -> matmul, densely
        vtb = None
        for (b, m, off, rn, idxi, w1, w2) in gate_info:
            if m == 0:
                vtb = vtpool.tile([128, KCH + 1, P], BF16)
            if True:
                # ---- gather the two expert rows
                G12 = gpool.tile([128, 2, SAUG], BF16, tag="G12")
                if MULTIROW_GATHER:
                    nc.gpsimd.indirect_dma_start(
                        out=G12[:rn], out_offset=None, in_=weff_d[:, :],
                        in_offset=IndirectOffsetOnAxis(ap=idxi[:rn, 0:2], axis=0),
                    )
                else:
                    nc.gpsimd.indirect_dma_start(
                        out=G12[:rn, 0, :], out_offset=None, in_=weff_d[:, :],
                        in_offset=IndirectOffsetOnAxis(ap=idxi[:rn, 0:1], axis=0),
                    )
                    nc.gpsimd.indirect_dma_start(
                        out=G12[:rn, 1, :], out_offset=None, in_=weff_d[:, :],
                        in_offset=IndirectOffsetOnAxis(ap=idxi[:rn, 1:2], axis=0),
                    )

                # ---- V = w1*G1 + w2*G2  (bf16)
                vtmp = vpool.tile([128, SAUG], BF16, tag="vtmp")
                nc.scalar.activation(vtmp[:rn], G12[:rn, 0, :], AF.Copy, scale=w1[:rn])
                V = vpool.tile([128, SAUG], BF16, tag="V")
                nc.vector.scalar_tensor_tensor(
                    V[:rn], G12[:rn, 1, :], w2[:rn], vtmp[:rn],
                    op0=ALU.mult, op1=ALU.add,
                )

                # ---- transpose V -> vtb[:, k, off:off+rn]; chunk 8 row0 = bias
                for k in range(KCH):
                    if XBAR_TRANSPOSE:
                        nc.scalar.dma_start_transpose(
                            out=vtb[:, k, off : off + rn],
                            in_=V[:rn, k * 128 : (k + 1) * 128],
                        )
                    else:
                        ptv = pp_tp.tile([128, 128], BF16, tag="tpb")
                        nc.tensor.transpose(
                            out=ptv[:, :rn],
                            in_=V[:rn, k * 128 : (k + 1) * 128],
                            identity=id_b[:rn, :rn],
                        )
                        nc.vector.tensor_copy(
                            vtb[:, k, off : off + rn], ptv[:, :rn]
                        )
                ptb = pp_tp.tile([128, 128], BF16, tag="tpb")
                nc.tensor.transpose(
                    out=ptb[:4, :rn], in_=V[:rn, S : S + 4],
                    identity=id_b[:rn, :rn],
                )
                nc.vector.tensor_copy(vtb[0:1, KCH, off : off + rn], ptb[0:1, :rn])

            # ---- main matmul: outT[f, tok] = x[b].T-chunks @ VT (+ bias row)
            if m != len(M_TILES) - 1:
                continue
            for fc in range(2):
                po = pp_o.tile([128, P], F32, tag="po")
                for k in range(KCH):
                    nc.tensor.matmul(
                        po, lhsT=xb16s[b][:, k, fc * 128 : (fc + 1) * 128],
                        rhs=vtb[:, k, :],
                        start=(k == 0), stop=False,
                    )
                nc.tensor.matmul(
                    po, lhsT=ones_bf[0:1, 0:128], rhs=vtb[0:1, KCH, :],
                    start=False, stop=True,
                )
                osb = opool.tile([128, P], F32, tag="osb")
                nc.vector.tensor_copy(osb, po)
                nc.sync.dma_start(
                    out_d[fc * 128 : (fc + 1) * 128, b * P : (b + 1) * P], osb
                )

        # ---- KL tail: klp = kl_scale * (sum slacc - E*sum ln zacc) + kl_bias
        ln24 = cpool.tile([128, NB * 3], F32)
        nc.scalar.activation(ln24, zacc, AF.Ln)
        kacc = cpool.tile([128, NB * 3], F32)
        nc.vector.scalar_tensor_tensor(
            kacc, ln24, -float(E), slacc, op0=ALU.mult, op1=ALU.add
        )
        kc = cpool.tile([128, 1], F32)
        nc.vector.tensor_reduce(kc, kacc, axis=mybir.AxisListType.X, op=ALU.add)
        pk = pp_lg.tile([1, 1], F32, tag="lg")
        nc.tensor.matmul(pk, lhsT=ones_c, rhs=kc, start=True, stop=True)
        kb = cpool.tile([1, 1], F32)
        nc.vector.memset(kb, kl_bias)
        ks = cpool.tile([1, 1], F32)
        nc.scalar.activation(ks, pk, AF.Identity, scale=kl_scale, bias=kb[:, :])
        nc.sync.dma_start(klp_d[:, :], ks)

    nc.compile()
    return nc


_CACHE = {}


def _prep_inputs(x, x_mark_enc, gate_w, gate_b, Wl, bl, Ws, bs):
    weff = fold_weights(Wl, bl, Ws, bs)
    gwt = np.zeros((F + 2 * NFREQ + 1, E), dtype=np.float32)
    gwt[: F + 2 * NFREQ, :] = gate_w.T
    gwt[F + 2 * NFREQ, :] = gate_b
    freqs = np.arange(1, NFREQ + 1, dtype=np.float32)
    cb8 = np.zeros((8, 2), dtype=np.float32)
    cb8[:4, 0] = freqs / (2.0 * MAX_TIME)
    cb8[4:, 0] = freqs / (2.0 * MAX_TIME)
    cb8[4:, 1] = 0.25
    hh = np.ascontiguousarray(x_mark_enc[:, S - P :, -1], dtype=np.float32)  # [B,P]
    in_maps = []
    for c in range(N_CORES):
        in_maps.append(
            {
                "x_l": np.ascontiguousarray(x[c * NB : (c + 1) * NB]),
                "hh_l": hh[c * NB : (c + 1) * NB].reshape(1, TOK),
                "weff": weff,
                "gwt": gwt,
                "cb8": cb8,
            }
        )
    return in_maps


def kernel(x, x_mark_enc, gate_w, gate_b, Wl, bl, Ws, bs, trace=False):
    if "nc" not in _CACHE:
        _CACHE["nc"] = build_module()
    nc = _CACHE["nc"]
    in_maps = _prep_inputs(x, x_mark_enc, gate_w, gate_b, Wl, bl, Ws, bs)
    res = bass_utils.run_bass_kernel_spmd(
        nc, in_maps, core_ids=list(range(N_CORES)), trace=trace
    )
    _CACHE["last_result"] = res
    out = np.concatenate(
        [np.ascontiguousarray(r["out_l"].T).reshape(NB, P, F) for r in res.results],
        axis=0,
    ).astype(np.float32)
    kl = np.float32(sum(float(r["klp"][0, 0]) for r in res.results))
    return out, kl


# revision 33
# speedup vs baseline: 2.9759x; 1.2142x over previous
"""Trainium2 Bass kernel for nn_MoELayer (moe_routing).

Strategy (data-parallel over batch, 8 NeuronCores):
  * Fold the multi-scale moving-average decomposition into the expert
    weights:  out_e[p,f] = sum_s x[s,f] * Weff[e,p,s] + bias[e,p] with
    Weff = Ws + sum_n A_n^T (Wl_n - Ws)  (A_n = reflect-pad moving avg).
    This shrinks the contraction 4096 -> 1024 and weight bytes 4x.
  * Exploit top-2-of-16 sparsity: compute the gate on device (fp32
    logits matmul -> max8/max_index -> w = sigmoid(dl)), indirect-DMA
    gather the two selected Weff rows per token (bf16), combine
    V = w1*G1 + w2*G2, PE-transpose to [S, tokens], then one bf16
    matmul per batch V @ x[b] accumulated in fp32 PSUM.
  * KL term computed on device from logits (sum log g = sum l - E*logZ),
    partial per core; host sums the 8 partial scalars.
"""

import math
import os
import sys

import numpy as np

for _p in ("/opt/trn_rl_repo",):
    if _p not in sys.path and os.path.isdir(_p):
        sys.path.append(_p)

import concourse.bass as bass
import concourse.mybir as mybir
from concourse import bacc
from concourse import bass_utils
from concourse.bass import IndirectOffsetOnAxis
from concourse.masks import make_identity
from concourse.tile import TileContext

F32 = mybir.dt.float32
BF16 = mybir.dt.bfloat16
I32 = mybir.dt.int32
U32 = mybir.dt.uint32
AF = mybir.ActivationFunctionType
ALU = mybir.AluOpType

# problem constants
B, S, F, E, P, nS = 64, 1024, 256, 16, 336, 3
SCALES = [3, 7, 14]
NFREQ = 4
MAX_TIME = 200.0
KL_LAMBDA = 1e-3
N_CORES = 8
NB = B // N_CORES          # batches per core
SAUG = S + 4               # weff rows padded: col 1024 = bias, 1025..1027 = 0
TOK = NB * P               # tokens per core (2688)
KCH = S // 128             # 8 contraction chunks
# token tiles per batch: p in [0,128), [128,256), [256,336)
M_TILES = [(0, 128), (128, 128), (256, 80)]
# x s-chunks holding the last P positions (s in [688, 1024)):
#   (chunk, col_offset_in_tokens, rows_used_from_chunk_top)
XT_CHUNKS = [(5, 0, 80), (6, 80, 128), (7, 208, 128)]
MULTIROW_GATHER = False
XBAR_TRANSPOSE = False


def _ma_matrix(n, w):
    """Dense [n,n] matrix of torch-style reflect-pad moving average."""
    lp = w // 2
    rp = lp - (1 if w % 2 == 0 else 0)
    A = np.zeros((n, n), dtype=np.float64)
    idx = np.zeros(n + lp + rp, dtype=np.int64)
    for j in range(n + lp + rp):
        if j < lp:
            idx[j] = lp - j
        elif j < lp + n:
            idx[j] = j - lp
        else:
            idx[j] = (n - 2) - (j - lp - n)
    inv = 1.0 / w
    for sp in range(n):
        for j in range(sp, sp + w):
            A[sp, idx[j]] += inv
    return A


def fold_weights(Wl, bl, Ws, bs):
    """Weff_aug [E*P, SAUG] bf16 (col S = bias, rest zero-pad)."""
    import ml_dtypes

    A = [_ma_matrix(S, w) for w in SCALES]
    Wsf = Ws.astype(np.float64)
    weff = np.array(Wsf)
    for n in range(nS):
        d = Wl[:, n, :, :].astype(np.float64) - Wsf
        # (A^T d)^T per row  ==  d @ A
        weff += (d.reshape(-1, S) @ A[n]).reshape(E, P, S)
    bias = (bl.sum(axis=1) + bs).astype(np.float64)
    aug = np.zeros((E * P, SAUG), dtype=np.float32)
    aug[:, :S] = weff.reshape(E * P, S).astype(np.float32)
    aug[:, S] = bias.reshape(E * P).astype(np.float32)
    return aug.astype(ml_dtypes.bfloat16)


def build_module():
    """Build the per-core Bass program (same program on all 8 cores)."""
    nc = bacc.Bacc("TRN2", target_bir_lowering=False)

    x_d = nc.dram_tensor("x_l", [NB, S, F], F32, kind="ExternalInput")
    hh_d = nc.dram_tensor("hh_l", [1, TOK], F32, kind="ExternalInput")
    weff_d = nc.dram_tensor("weff", [E * P, SAUG], BF16, kind="ExternalInput")
    gwt_d = nc.dram_tensor("gwt", [F + 2 * NFREQ + 1, E], F32, kind="ExternalInput")
    cb8_d = nc.dram_tensor("cb8", [128, 2], F32, kind="ExternalInput")
    out_d = nc.dram_tensor("out_l", [F, TOK], F32, kind="ExternalOutput")
    klp_d = nc.dram_tensor("klp", [1, 1], F32, kind="ExternalOutput")
    encs_d = nc.dram_tensor("encs", [8, TOK], F32)  # internal scratch

    u = 1.0 / E
    kl_scale = -KL_LAMBDA * u / B
    kl_bias = KL_LAMBDA * u * (TOK * E) * math.log(u) / B

    from contextlib import ExitStack

    with TileContext(nc) as tc, ExitStack() as es:
        cpool = es.enter_context(tc.tile_pool(name="const", bufs=1))
        spool = es.enter_context(tc.tile_pool(name="smalls", bufs=3))
        wpool = es.enter_context(tc.tile_pool(name="wcoef", bufs=26))
        xfpool = es.enter_context(tc.tile_pool(name="xf", bufs=2))
        xbpool = es.enter_context(tc.tile_pool(name="xb", bufs=1))
        xtpool = es.enter_context(tc.tile_pool(name="xt", bufs=2))
        gpool = es.enter_context(tc.tile_pool(name="gath", bufs=4))
        vpool = es.enter_context(tc.tile_pool(name="vcomb", bufs=3))
        vtpool = es.enter_context(tc.tile_pool(name="vt", bufs=3))
        opool = es.enter_context(tc.tile_pool(name="osb", bufs=3))
        pp_tp = es.enter_context(tc.tile_pool(name="ps_tp", bufs=2, space="PSUM"))
        pp_lg = es.enter_context(tc.tile_pool(name="ps_lg", bufs=1, space="PSUM"))
        pp_o = es.enter_context(tc.tile_pool(name="ps_o", bufs=2, space="PSUM"))

        # ---- constants
        id_f = cpool.tile([128, 128], F32)
        make_identity(nc, id_f)
        id_b = cpool.tile([128, 128], BF16)
        make_identity(nc, id_b)
        gw0 = cpool.tile([128, E], F32)
        nc.sync.dma_start(gw0, gwt_d[0:128, :])
        gw1 = cpool.tile([128, E], F32)
        nc.sync.dma_start(gw1, gwt_d[128:256, :])
        gw2 = cpool.tile([9, E], F32)
        nc.sync.dma_start(gw2, gwt_d[256:265, :])
        cb128 = cpool.tile([128, 2], F32)
        nc.sync.dma_start(cb128, cb8_d[:, :])
        ones_c = cpool.tile([128, 1], F32)
        nc.vector.memset(ones_c, 1.0)
        ones_bf = cpool.tile([1, 128], BF16)
        nc.vector.memset(ones_bf, 1.0)
        # KL accumulators: Z per (b,m) column (init 1 -> ln=0), sum-of-logits
        zacc = cpool.tile([128, NB * 3], F32)
        nc.vector.memset(zacc, 1.0)
        slacc = cpool.tile([128, NB * 3], F32)
        nc.vector.memset(slacc, 0.0)
        pcol = []
        for m, (off, rn) in enumerate(M_TILES):
            pi = cpool.tile([128, 1], I32, tag=f"pci{m}")
            nc.gpsimd.iota(pi, pattern=[[0, 1]], base=off, channel_multiplier=1)
            pf = cpool.tile([128, 1], F32, tag=f"pcf{m}")
            nc.vector.tensor_copy(pf, pi)
            pcol.append(pf)

        # ---- stage encoding (range-reduced sin), fp32, all tokens at once.
        # Work in a [128, TOK/16] layout (partition = channel*16 + group) so
        # DVE ops are 16x faster than the natural [8, TOK] layout; tiny
        # SBUF->SBUF DMAs reshape to enc9 rows [8+1, TOK] at the end.
        # u = t*(f/2) (+0.25 for cos channels); v = u mod 1; a = v - (v>=.5)
        # enc = sin(2*pi*a)
        TG = TOK // 16
        enc9 = cpool.tile([9, TOK], F32)
        nc.vector.memset(enc9, 1.0)
        u128 = cpool.tile([128, TG], F32)
        for ch in range(8):
            nc.sync.dma_start(
                u128[ch * 16 : (ch + 1) * 16, :],
                hh_d[:, :].rearrange("o (g t) -> (o g) t", g=16),
            )
        nc.scalar.activation(
            u128, u128, AF.Identity, scale=cb128[:, 0:1], bias=cb128[:, 1:2]
        )
        m1 = cpool.tile([128, TG], F32, tag="enctmp")
        nc.vector.tensor_scalar(m1, u128, 1.0, scalar2=None, op0=ALU.is_ge)
        nc.vector.tensor_sub(u128, u128, m1)
        nc.vector.tensor_scalar(m1, u128, 1.0, scalar2=None, op0=ALU.is_ge)
        nc.vector.tensor_sub(u128, u128, m1)
        nc.vector.tensor_scalar(m1, u128, 0.5, scalar2=None, op0=ALU.is_ge)
        nc.vector.tensor_sub(u128, u128, m1)
        nc.scalar.activation(u128, u128, AF.Sin, scale=2.0 * math.pi)
        # reshape [128, TG] -> [8, TOK] via a DRAM bounce (SBUF partition dims
        # cannot be flattened in an SBUF-side AP)
        nc.sync.dma_start(
            encs_d[:, :].rearrange("c (g t) -> (c g) t", g=16), u128
        )
        nc.sync.dma_start(enc9[0:8, :], encs_d[:, :])

        # ---- pass 1: x load/cast + gating for all batches
        xb16s = []
        gate_info = []  # (b, m, off, rn, idxi, w1, w2)
        for b in range(NB):
            xf = xfpool.tile([128, KCH, F], F32)
            nc.sync.dma_start(
                xf, x_d[b : b + 1, :, :].rearrange("o (k p) f -> p (o k) f", p=128)
            )
            xb16 = xbpool.tile([128, KCH, F], BF16, tag=f"xb{b}")
            nc.vector.tensor_copy(xb16, xf)
            xb16s.append(xb16)

            # transpose the gating slice of x: xt[f, fc, tokens]
            xt = xtpool.tile([128, 2, P], F32)
            for (kch, c0, rows) in XT_CHUNKS:
                for fc in range(2):
                    pt = pp_tp.tile([128, 128], F32, tag="tp")
                    nc.tensor.transpose(
                        out=pt,
                        in_=xf[:, kch, fc * 128 : (fc + 1) * 128],
                        identity=id_f,
                    )
                    nc.vector.tensor_copy(
                        xt[:, fc, c0 : c0 + rows], pt[:, 128 - rows : 128]
                    )

            for m, (off, rn) in enumerate(M_TILES):
                # ---- logits (fp32, exact)
                pl = pp_lg.tile([128, E], F32, tag="lg")
                nc.tensor.matmul(
                    pl[:rn], lhsT=xt[:, 0, off : off + rn], rhs=gw0,
                    start=True, stop=False,
                )
                nc.tensor.matmul(
                    pl[:rn], lhsT=xt[:, 1, off : off + rn], rhs=gw1,
                    start=False, stop=False,
                )
                nc.tensor.matmul(
                    pl[:rn], lhsT=enc9[:, b * P + off : b * P + off + rn], rhs=gw2,
                    start=False, stop=True,
                )
                L = spool.tile([128, E], F32, tag="L")
                nc.vector.tensor_copy(L[:rn], pl[:rn])

                # ---- KL pieces (Ln deferred to the tail; one table set here)
                col = b * 3 + m
                eZ = spool.tile([128, E], F32, tag="eZ")
                nc.scalar.activation(
                    eZ[:rn], L[:rn], AF.Exp, accum_out=zacc[:rn, col : col + 1]
                )
                nc.vector.tensor_reduce(
                    slacc[:rn, col : col + 1], L[:rn],
                    axis=mybir.AxisListType.X, op=ALU.add,
                )

                # ---- top-2; gate weights via exp ratio (no sigmoid table)
                M8 = spool.tile([128, 8], F32, tag="M8")
                nc.vector.max(M8[:rn], L[:rn])
                I8 = spool.tile([128, 8], U32, tag="I8")
                nc.vector.max_index(I8[:rn], M8[:rn], L[:rn])
                E2 = spool.tile([128, 2], F32, tag="E2")
                nc.scalar.activation(E2[:rn], M8[:rn, 0:2], AF.Exp)
                s12 = spool.tile([128, 1], F32, tag="s12")
                nc.vector.tensor_add(s12[:rn], E2[:rn, 0:1], E2[:rn, 1:2])
                r12 = spool.tile([128, 1], F32, tag="r12")
                nc.vector.reciprocal(r12[:rn], s12[:rn])
                w1 = wpool.tile([128, 1], F32, tag="w1")
                nc.vector.tensor_mul(w1[:rn], E2[:rn, 0:1], r12[:rn])
                w2 = wpool.tile([128, 1], F32, tag="w2")
                nc.vector.tensor_mul(w2[:rn], E2[:rn, 1:2], r12[:rn])

                # ---- weff row ids: idx = e*P + p
                If2 = spool.tile([128, 2], F32, tag="If2")
                nc.vector.tensor_copy(If2[:rn], I8[:rn, 0:2])
                idxf = spool.tile([128, 2], F32, tag="idxf")
                nc.vector.scalar_tensor_tensor(
                    idxf[:rn], If2[:rn], float(P),
                    pcol[m].to_broadcast([128, 2])[:rn], op0=ALU.mult, op1=ALU.add,
                )
                idxi = wpool.tile([128, 2], I32, tag="idxi")
                nc.vector.tensor_copy(idxi[:rn], idxf[:rn])
                gate_info.append((b, m, off, rn, idxi, w1, w2))

        # ---- pass 2: gather -> combine -> transpose -> matmul, densely
        vtb = None
        for (b, m, off, rn, idxi, w1, w2) in gate_info:
            if m == 0:
                vtb = vtpool.tile([128, KCH + 1, P], BF16)
            if True:
                # ---- gather the two expert rows
                G12 = gpool.tile([128, 2, SAUG], BF16, tag="G12")
                if MULTIROW_GATHER:
                    nc.gpsimd.indirect_dma_start(
                        out=G12[:rn], out_offset=None, in_=weff_d[:, :],
                        in_offset=IndirectOffsetOnAxis(ap=idxi[:rn, 0:2], axis=0),
                    )
                else:
                    nc.gpsimd.indirect_dma_start(
                        out=G12[:rn, 0, :], out_offset=None, in_=weff_d[:, :],
                        in_offset=IndirectOffsetOnAxis(ap=idxi[:rn, 0:1], axis=0),
                    )
                    nc.gpsimd.indirect_dma_start(
                        out=G12[:rn, 1, :], out_offset=None, in_=weff_d[:, :],
                        in_offset=IndirectOffsetOnAxis(ap=idxi[:rn, 1:2], axis=0),
                    )

                # ---- V = w1*G1 + w2*G2  (bf16)
                vtmp = vpool.tile([128, SAUG], BF16, tag="vtmp")
                nc.scalar.activation(vtmp[:rn], G12[:rn, 0, :], AF.Copy, scale=w1[:rn])
                V = vpool.tile([128, SAUG], BF16, tag="V")
                nc.vector.scalar_tensor_tensor(
                    V[:rn], G12[:rn, 1, :], w2[:rn], vtmp[:rn],
                    op0=ALU.mult, op1=ALU.add,
                )

                # ---- transpose V -> vtb[:, k, off:off+rn]; chunk 8 row0 = bias
                # 4 transposes share one PSUM tile -> one batched DVE copy
                for k4 in range(0, KCH, 4):
                    ptv = pp_tp.tile([128, 4, 128], BF16, tag="tp4")
                    for k in range(k4, k4 + 4):
                        nc.tensor.transpose(
                            out=ptv[:, k - k4, :rn],
                            in_=V[:rn, k * 128 : (k + 1) * 128],
                            identity=id_b[:rn, :rn],
                        )
                    nc.vector.tensor_copy(
                        vtb[:, k4 : k4 + 4, off : off + rn], ptv[:, :, :rn]
                    )
                ptb = pp_tp.tile([128, 128], BF16, tag="tp4")
                nc.tensor.transpose(
                    out=ptb[:4, :rn], in_=V[:rn, S : S + 4],
                    identity=id_b[:rn, :rn],
                )
                nc.vector.tensor_copy(vtb[0:1, KCH, off : off + rn], ptb[0:1, :rn])

            # ---- main matmul: outT[f, tok] = x[b].T-chunks @ VT (+ bias row)
            if m != len(M_TILES) - 1:
                continue
            for fc in range(2):
                po = pp_o.tile([128, P], F32, tag="po")
                for k in range(KCH):
                    nc.tensor.matmul(
                        po, lhsT=xb16s[b][:, k, fc * 128 : (fc + 1) * 128],
                        rhs=vtb[:, k, :],
                        start=(k == 0), stop=False,
                    )
                nc.tensor.matmul(
                    po, lhsT=ones_bf[0:1, 0:128], rhs=vtb[0:1, KCH, :],
                    start=False, stop=True,
                )
                osb = opool.tile([128, P], F32, tag="osb")
                nc.vector.tensor_copy(osb, po)
                nc.sync.dma_start(
                    out_d[fc * 128 : (fc + 1) * 128, b * P : (b + 1) * P], osb
                )

        # ---- KL tail: klp = kl_scale * (sum slacc - E*sum ln zacc) + kl_bias
        ln24 = cpool.tile([128, NB * 3], F32)
        nc.scalar.activation(ln24, zacc, AF.Ln)
        kacc = cpool.tile([128, NB * 3], F32)
        nc.vector.scalar_tensor_tensor(
            kacc, ln24, -float(E), slacc, op0=ALU.mult, op1=ALU.add
        )
        kc = cpool.tile([128, 1], F32)
        nc.vector.tensor_reduce(kc, kacc, axis=mybir.AxisListType.X, op=ALU.add)
        pk = pp_lg.tile([1, 1], F32, tag="lg")
        nc.tensor.matmul(pk, lhsT=ones_c, rhs=kc, start=True, stop=True)
        kb = cpool.tile([1, 1], F32)
        nc.vector.memset(kb, kl_bias)
        ks = cpool.tile([1, 1], F32)
        nc.scalar.activation(ks, pk, AF.Identity, scale=kl_scale, bias=kb[:, :])
        nc.sync.dma_start(klp_d[:, :], ks)

    nc.compile()
    return nc


_CACHE = {}


def _prep_inputs(x, x_mark_enc, gate_w, gate_b, Wl, bl, Ws, bs):
    weff = fold_weights(Wl, bl, Ws, bs)
    gwt = np.zeros((F + 2 * NFREQ + 1, E), dtype=np.float32)
    gwt[: F + 2 * NFREQ, :] = gate_w.T
    gwt[F + 2 * NFREQ, :] = gate_b
    freqs = np.arange(1, NFREQ + 1, dtype=np.float32)
    cb8 = np.zeros((128, 2), dtype=np.float32)
    # partition = channel*16 + group; channels 0-3 sin(f), 4-7 cos(f)
    for ch in range(8):
        cb8[ch * 16 : (ch + 1) * 16, 0] = freqs[ch % 4] / (2.0 * MAX_TIME)
        if ch >= 4:
            cb8[ch * 16 : (ch + 1) * 16, 1] = 0.25
    hh = np.ascontiguousarray(x_mark_enc[:, S - P :, -1], dtype=np.float32)  # [B,P]
    in_maps = []
    for c in range(N_CORES):
        in_maps.append(
            {
                "x_l": np.ascontiguousarray(x[c * NB : (c + 1) * NB]),
                "hh_l": hh[c * NB : (c + 1) * NB].reshape(1, TOK),
                "weff": weff,
                "gwt": gwt,
                "cb8": cb8,
            }
        )
    return in_maps


def kernel(x, x_mark_enc, gate_w, gate_b, Wl, bl, Ws, bs, trace=False):
    if "nc" not in _CACHE:
        _CACHE["nc"] = build_module()
    nc = _CACHE["nc"]
    in_maps = _prep_inputs(x, x_mark_enc, gate_w, gate_b, Wl, bl, Ws, bs)
    res = bass_utils.run_bass_kernel_spmd(
        nc, in_maps, core_ids=list(range(N_CORES)), trace=trace
    )
    _CACHE["last_result"] = res
    out = np.concatenate(
        [np.ascontiguousarray(r["out_l"].T).reshape(NB, P, F) for r in res.results],
        axis=0,
    ).astype(np.float32)
    kl = np.float32(sum(float(r["klp"][0, 0]) for r in res.results))
    return out, kl
